# revision 36
# baseline (speedup 1.0000x reference)
"""2-layer GCN (100k nodes, 3.2M edges) on 8 Trainium2 NeuronCores.

Strategy (graph/data parallel, per the node-partition + halo-exchange hint):
  - Nodes are range-partitioned across the 8 cores (12500 each + 44 dummies
    -> 12544 = 98*128 positions per core).
  - GCN algebra: out = D^-1/2 A_hat D^-1/2 (H W).  We pre-scale each node's
    transformed features by dinv, segment-sum over in-edges, and post-scale
    by dinv; for layer 2 we aggregate first and apply W2 after (linearity),
    so both layers aggregate 16-dim features.
  - Per layer, each core computes its shard of the (scaled) feature table,
    the shards are AllGather'd (the halo exchange: feature-major [16, 12544]
    f32 per core -> [128, 12544] global table in DRAM).
  - Aggregation: the SBUF table is REPLICATED 2x: each 16-partition GPSIMD
    group g holds shard g (cols [0,NPC)) and shard (g+1)%8 (cols [NPC,2NPC)).
    Every edge therefore has TWO candidate groups; a host-side 2-choice
    balancing pass assigns edges to groups so the per-(dst, group) counts are
    nearly even (an exact cycle-balancing: max load hits the arc lower
    bound), cutting the max-over-groups slot padding from ~8.3 to ~4.7 per
    node (the ap_gather ucode cost is per-slot, ~28ns, so fewer slots is
    the whole ballgame).  Edges' source features are fetched with the
    ap_gather ucode; a DVE segmented reduce (per-block widths) produces
    per-group partial sums; a PE matmul against a replicated selector
    (layer 1) or replicated W2 (layer 2) sums across the 8 groups.

All floating-point arithmetic (matmuls, degree->rsqrt, aggregation, bias,
relu, log_softmax) runs on device.  The host only restructures integers
(edge lists -> per-block index tensors) and permutes/relayouts tensors.
"""

import numpy as np

import concourse.bass as bass
import concourse.bacc as bacc
import concourse.mybir as mybir
import concourse.tile as tile
from concourse.bass_utils import run_bass_kernel_spmd

N_NODES = 100000
N_FEAT = 512
HIDDEN = 16
N_CLASSES = 64
NCORES = 8
NPC_REAL = 12500          # real nodes per core
NPC = 12544               # padded positions per core (98 * 128)
NBLK = NPC // 128         # 98 blocks of 128 nodes
SB = 8                    # max blocks per super-block (ap_gather/reduce batch)
DUMMY_COL = NPC - 1       # every shard's last position is a dummy (zero) node

_cache = {}


# ----------------------------------------------------------------------------
# host-side graph restructuring (integer work only)
# ----------------------------------------------------------------------------

def _two_choice_split(c):
    """Optimally balance per-dst counts over the 8-group cycle.

    Bucket (d, s) holds c[d, s] edges; x[d, s] of them go to group s, the
    rest to group (s-1) % 8, giving load[d, g] = x[d, g] + c[d, g+1] -
    x[d, g+1].  The optimal max-load is the cycle-arc bound
    max over arcs [i..i+L) of ceil(sum c / (L+1)); a minimal-x greedy
    propagation around the cycle achieves it.
    """
    c = c.astype(np.int64)
    n = c.shape[0]
    # arc lower bound (L buckets spread over L+1 groups; L=8 is the full cycle)
    M = -(-c.sum(axis=1) // 8)
    for L in range(1, 8):
        for s0 in range(8):
            tot = c[:, [(s0 + k) % 8 for k in range(L)]].sum(axis=1)
            M = np.maximum(M, -(-tot // (L + 1)))
    # least solution of the cyclic difference constraints
    # x_g >= x_{g-1} + c_g - M, x >= 0:
    #   x_g = max(0, max_{1<=k<=7} sum_{j=g-k+1..g} c_j - k*M)
    # (achieves load <= M exactly; x <= c follows from the arc bounds)
    X = np.zeros((n, 8), dtype=np.int64)
    for g in range(8):
        best = np.zeros(n, dtype=np.int64)
        acc = np.zeros(n, dtype=np.int64)
        for k in range(1, 8):
            acc = acc + c[:, (g - k + 1) % 8]
            best = np.maximum(best, acc - k * M)
        X[:, g] = best
    load = X + np.roll(c, -1, axis=1) - np.roll(X, -1, axis=1)
    return X, load


def _preprocess(edge_index):
    src = edge_index[0].astype(np.int64)
    dst = edge_index[1].astype(np.int64)

    # in-degree INCLUDES the self-loop; but self-loop edges are handled
    # locally (shard add), not gathered, so they are excluded from the slots
    deg = np.bincount(dst, minlength=N_NODES) + 1

    owner = src // NPC_REAL

    c = np.bincount(dst * 8 + owner, minlength=N_NODES * 8).reshape(N_NODES, 8)
    x, load = _two_choice_split(c)                                # per-group loads
    dtil = load.max(axis=1)                                       # slots per node

    # per-edge group assignment: within bucket (d, s) the first x[d, s]
    # edges (stable order) stay in group s, the rest go to group (s-1) % 8
    key = dst * 8 + owner
    perm = np.argsort(key, kind="stable")
    key_s = key[perm]
    starts = np.zeros(N_NODES * 8 + 1, dtype=np.int64)
    starts[1:] = np.cumsum(c.ravel())
    j_bucket = np.arange(len(key_s), dtype=np.int64) - starts[key_s]
    own_s = key_s % 8
    stay = j_bucket < x.ravel()[key_s]
    grp_s = np.where(stay, own_s, (own_s - 1) % 8)

    src_s = src[perm]
    dst_s = dst[perm]

    # per-core permutation: sort local nodes by dtil desc; dummies last
    order = np.empty((NCORES, NPC), dtype=np.int64)   # position -> local node id
    rank = np.empty(N_NODES, dtype=np.int64)          # global node -> position
    for cc in range(NCORES):
        lo = cc * NPC_REAL
        d_loc = np.concatenate(
            [dtil[lo : lo + NPC_REAL], np.full(NPC - NPC_REAL, -1, np.int64)]
        )
        o = np.argsort(-d_loc, kind="stable")
        order[cc] = o
        inv = np.empty(NPC, dtype=np.int64)
        inv[o] = np.arange(NPC)
        rank[lo : lo + NPC_REAL] = inv[:NPC_REAL]

    # block widths, unified across cores (SPMD shares the program)
    dtil_pos = np.zeros((NCORES, NPC), dtype=np.int64)
    for cc in range(NCORES):
        lo = cc * NPC_REAL
        real = order[cc] < NPC_REAL
        dtil_pos[cc][real] = dtil[lo + order[cc][real]]
    blk_D = np.zeros(NBLK, dtype=np.int64)
    for b in range(NBLK):
        blk_D[b] = max(1, dtil_pos[:, b * 128 : (b + 1) * 128].max())

    NI_CAP = max(3584, int(128 * blk_D.max()))
    # supers: consecutive blocks, per-block widths, sum(128*D_b) <= NI_CAP
    supers = []  # (b0, nblk, [D_b...], [col base of each block])
    b = 0
    while b < NBLK:
        nb = 0
        tot = 0
        cbs = []
        while b + nb < NBLK and nb < SB and tot + 128 * blk_D[b + nb] <= NI_CAP:
            cbs.append(tot)
            tot += 128 * int(blk_D[b + nb])
            nb += 1
        supers.append((b, nb, [int(blk_D[b + k]) for k in range(nb)], cbs))
        b += nb
    n_super = len(supers)
    sup_of_blk = np.zeros(NBLK, dtype=np.int64)
    blk_cb = np.zeros(NBLK, dtype=np.int64)       # col base within its super
    sup_b0 = np.zeros(NBLK, dtype=np.int64)
    for si, (b0, nb, Ds, cbs) in enumerate(supers):
        for k in range(nb):
            sup_of_blk[b0 + k] = si
            blk_cb[b0 + k] = cbs[k]
            sup_b0[b0 + k] = b0
    num_idxs = np.array(
        [sum(128 * D for D in Ds) for (_, _, Ds, _) in supers], dtype=np.int64
    )
    colbase = np.zeros(n_super + 1, dtype=np.int64)
    colbase[1:] = np.cumsum(num_idxs // 16)
    IDXCOLS = int(colbase[-1])

    # per-edge slot assignment within the assigned group
    key2 = dst_s * 8 + grp_s
    perm2 = np.argsort(key2, kind="stable")
    key2_s = key2[perm2]
    src2 = src_s[perm2]
    grp2 = grp_s[perm2]
    own2 = own_s[perm2]
    cnt2 = np.bincount(key2_s, minlength=N_NODES * 8)
    starts2 = np.zeros(N_NODES * 8 + 1, dtype=np.int64)
    starts2[1:] = np.cumsum(cnt2)
    j_within = np.arange(len(key2_s), dtype=np.int64) - starts2[key2_s]

    dst2 = key2_s // 8
    c2 = dst2 // NPC_REAL                              # dst's core
    pos2 = rank[dst2]                                  # position within core
    blk2 = pos2 // 128
    i2 = pos2 % 128
    DB = blk_D[blk2]
    e_col = blk_cb[blk2] + i2 * DB + j_within          # col within instruction
    part = 16 * grp2 + (e_col % 16)
    col = colbase[sup_of_blk[blk2]] + e_col // 16
    # table column of the source within group grp2's two shard halves
    val = rank[src2] + NPC * (own2 != grp2)

    idx_all = np.full((NCORES, 128, IDXCOLS), DUMMY_COL, dtype=np.int16)
    idx_all[c2, part, col] = val.astype(np.int16)

    # per-core degree tensors in (partition, block) layout
    deg_pb = np.zeros((NCORES, 128, NBLK), dtype=np.int32)
    for cc in range(NCORES):
        lo = cc * NPC_REAL
        real = order[cc] < NPC_REAL
        d = np.zeros(NPC, dtype=np.int32)
        d[real] = deg[lo + order[cc][real]].astype(np.int32)
        deg_pb[cc] = d.reshape(NBLK, 128).T            # pos = b*128 + p
    deg_rep = np.repeat(deg_pb, HIDDEN, axis=2).reshape(NCORES, 128, NBLK * HIDDEN)

    return {
        "order": order,
        "idx_all": idx_all,
        "deg_rep": deg_rep,
        "supers": supers,
        "num_idxs": num_idxs,
        "colbase": colbase,
        "IDXCOLS": IDXCOLS,
        "NI_CAP": NI_CAP,
    }


# ----------------------------------------------------------------------------
# device program
# ----------------------------------------------------------------------------

def _build_program(meta):
    supers = meta["supers"]
    num_idxs = meta["num_idxs"]
    colbase = meta["colbase"]
    IDXCOLS = meta["IDXCOLS"]
    NI_CAP = meta["NI_CAP"]
    PH_SB = 4                          # phase-A block group size
    n_phA = (NBLK + PH_SB - 1) // PH_SB
    f32 = mybir.dt.float32

    nc = bacc.Bacc(
        "TRN2", target_bir_lowering=False, debug=False, num_devices=NCORES
    )
    xT = nc.declare_dram_parameter("xT", [N_FEAT, NPC], f32, isOutput=False)
    idx_in = nc.declare_dram_parameter(
        "idx_in", [128, IDXCOLS], mybir.dt.int16, isOutput=False
    )
    degrep_in = nc.declare_dram_parameter(
        "degrep_in", [128, NBLK * HIDDEN], mybir.dt.int32, isOutput=False
    )
    W1r_in = nc.declare_dram_parameter("W1r", [128, 64], f32, isOutput=False)
    b1r_in = nc.declare_dram_parameter("b1r", [128, SB * HIDDEN], f32, isOutput=False)
    E8I_in = nc.declare_dram_parameter("E8I", [128, HIDDEN], f32, isOutput=False)
    W2r_in = nc.declare_dram_parameter("W2r", [128, N_CLASSES], f32, isOutput=False)
    b2r_in = nc.declare_dram_parameter(
        "b2r", [128, SB * N_CLASSES], f32, isOutput=False
    )
    ident_in = nc.declare_dram_parameter("ident", [128, 128], f32, isOutput=False)
    dmask_in = nc.declare_dram_parameter("dmask", [128, 1], f32, isOutput=False)
    out_d = nc.declare_dram_parameter("out", [NBLK, 128, N_CLASSES], f32, isOutput=True)

    q1d = nc.dram_tensor("q1d", [16, NPC], f32)
    q2d = nc.dram_tensor("q2d", [16, NPC], f32)
    tab1d = nc.dram_tensor("tab1d", [128, NPC], f32, addr_space="Shared")
    tab2d = nc.dram_tensor("tab2d", [128, NPC], f32, addr_space="Shared")
    warm_i = nc.dram_tensor("warm_i", [16, 16], f32)
    warm_o = nc.dram_tensor("warm_o", [128, 16], f32, addr_space="Shared")

    rg = [list(range(NCORES))]

    with tile.TileContext(nc) as tc:
        with (
            tc.tile_pool(name="const", bufs=1) as cp,
            tc.tile_pool(name="xt", bufs=2) as xp,
            tc.tile_pool(name="msg", bufs=2) as mp,
            tc.tile_pool(name="work", bufs=2) as wp,
            tc.tile_pool(name="selfq", bufs=2) as sq,
            tc.tile_pool(name="tab", bufs=1) as tp,
            tc.tile_pool(name="ps", bufs=2, space="PSUM") as pp,
            tc.tile_pool(name="psT", bufs=2, space="PSUM") as ppT,
            tc.tile_pool(name="psO", bufs=2, space="PSUM") as ppO,
        ):
            # ---- warmup collective: rendezvous the 8 cores NOW so the real
            # allgather later doesn't absorb the kernel-launch skew ----------
            nc.gpsimd.collective_compute(
                "AllGather",
                mybir.AluOpType.bypass,
                replica_groups=rg,
                ins=[warm_i[:]],
                outs=[warm_o[:]],
            )

            # ---- constants -------------------------------------------------
            W1r = cp.tile([128, 64], f32)
            nc.sync.dma_start(out=W1r[:], in_=W1r_in[:])
            b1r = cp.tile([128, SB * HIDDEN], f32)
            nc.sync.dma_start(out=b1r[:], in_=b1r_in[:])
            E8I = cp.tile([128, HIDDEN], f32)
            nc.sync.dma_start(out=E8I[:], in_=E8I_in[:])
            W2r = cp.tile([128, N_CLASSES], f32)
            nc.sync.dma_start(out=W2r[:], in_=W2r_in[:])
            b2r = cp.tile([128, SB * N_CLASSES], f32)
            nc.sync.dma_start(out=b2r[:], in_=b2r_in[:])
            ident = cp.tile([128, 128], f32)
            nc.sync.dma_start(out=ident[:], in_=ident_in[:])
            dmask = cp.tile([128, 1], f32)
            nc.sync.dma_start(out=dmask[:], in_=dmask_in[:])
            idx_sb = cp.tile([128, IDXCOLS], mybir.dt.int16)
            nc.sync.dma_start(out=idx_sb[:], in_=idx_in[:])

            # dinv (repeated 16x per block): rsqrt(max(deg,1)) on device
            degrep = cp.tile([128, NBLK * HIDDEN], mybir.dt.int32)
            nc.sync.dma_start(out=degrep[:], in_=degrep_in[:])
            dinvr = cp.tile([128, NBLK * HIDDEN], f32)
            nc.vector.tensor_copy(out=dinvr[:], in_=degrep[:])
            nc.vector.tensor_scalar_max(out=dinvr[:], in0=dinvr[:], scalar1=1.0)
            nc.vector.reciprocal(out=dinvr[:], in_=dinvr[:])
            nc.scalar.activation(
                out=dinvr[:], in_=dinvr[:], func=mybir.ActivationFunctionType.Sqrt
            )

            table = tp.tile([128, 2 * NPC], f32)  # 2x-replicated global table

            def load_table(tabd):
                """group g rows: shard g at cols [0,NPC), shard (g+1)%8 at
                cols [NPC, 2*NPC).  Split across both HWDGE engines."""
                nc.sync.dma_start(out=table[:, :NPC], in_=tabd[:])
                nc.scalar.dma_start(out=table[0:112, NPC:], in_=tabd[16:128, :])
                nc.scalar.dma_start(out=table[112:128, NPC:], in_=tabd[0:16, :])

            def post_to_dram(qa4, b0, nblk_s, qdram):
                """transpose node-major [128, nblk_s*16] -> DRAM shard strips."""
                for j in range(nblk_s):
                    b = b0 + j
                    psT = ppT.tile([HIDDEN, 128], f32, tag="psT")
                    nc.tensor.transpose(
                        out=psT[:],
                        in_=qa4[:, j * HIDDEN : (j + 1) * HIDDEN],
                        identity=ident[:],
                    )
                    strip = wp.tile([HIDDEN, 128], f32, tag="strip")
                    nc.vector.tensor_copy(out=strip[:], in_=psT[:])
                    nc.sync.dma_start(
                        out=qdram[:, b * 128 : (b + 1) * 128], in_=strip[:]
                    )

            # ---- phase A: q1 = (x @ W1) * dinv, computed feature-major -----
            # lhsT is the tiny W1 chunk [128, 16]; the 512-node x chunk is
            # STREAMED, so the PE does one weight load per chunk instead of
            # one 128x128 load per block, and the output [16, nodes] is
            # already feature-major (no transposes).
            for s in range(n_phA):
                if s == n_phA // 2 + 1:
                    # second rendezvous: cores re-drift ~18us across phase A;
                    # re-sync here (gpsimd queue is idle, so this hides
                    # completely) so AG1's wait shrinks
                    nc.gpsimd.collective_compute(
                        "AllGather",
                        mybir.AluOpType.bypass,
                        replica_groups=rg,
                        ins=[warm_i[:]],
                        outs=[warm_o[:]],
                    )
                b0 = s * PH_SB
                nblk_s = min(PH_SB, NBLK - b0)
                w = nblk_s * 128
                xts = []
                for kc in range(4):
                    xt = xp.tile([128, PH_SB * 128], f32, tag=f"xt{kc}")
                    nc.sync.dma_start(
                        out=xt[:, :w],
                        in_=xT[kc * 128 : (kc + 1) * 128, b0 * 128 : b0 * 128 + w],
                    )
                    xts.append(xt)
                psF = pp.tile([HIDDEN, PH_SB * 128], f32, tag="psF")
                for kc in range(4):
                    nc.tensor.matmul(
                        out=psF[:, :w],
                        lhsT=W1r[:, kc * HIDDEN : (kc + 1) * HIDDEN],
                        rhs=xts[kc][:, :w],
                        start=(kc == 0),
                        stop=(kc == 3),
                    )
                # dinvr's block slice [128, 16] is the same dinv value
                # replicated across the 16 feature columns, so its transpose
                # IS the feature-major [16, 128] dinv tile for the block.
                strip = wp.tile([HIDDEN, PH_SB * 128], f32, tag="stripA")
                for j in range(nblk_s):
                    b = b0 + j
                    psD = ppT.tile([HIDDEN, 128], f32, tag="psT")
                    nc.tensor.transpose(
                        out=psD[:],
                        in_=dinvr[:, b * HIDDEN : (b + 1) * HIDDEN],
                        identity=ident[:],
                    )
                    sbD = wp.tile([HIDDEN, 128], f32, tag="sbD")
                    nc.vector.tensor_copy(out=sbD[:], in_=psD[:])
                    nc.vector.tensor_tensor(
                        out=strip[:, j * 128 : (j + 1) * 128],
                        in0=psF[:, j * 128 : (j + 1) * 128],
                        in1=sbD[:],
                        op=mybir.AluOpType.mult,
                    )
                nc.sync.dma_start(
                    out=q1d[:, b0 * 128 : b0 * 128 + w], in_=strip[:, :w]
                )

            # ---- allgather 1 + table load ---------------------------------
            nc.gpsimd.collective_compute(
                "AllGather",
                mybir.AluOpType.bypass,
                replica_groups=rg,
                ins=[q1d[:]],
                outs=[tab1d[:]],
            )
            load_table(tab1d)

            # ---- aggregation helper ---------------------------------------
            def aggregate(s, qdram):
                """gather + per-block segmented reduce -> [128, nodes]."""
                b0, nblk_s, Ds, cbs = supers[s]
                nodes = nblk_s * 128
                ni = int(num_idxs[s])
                selfq = sq.tile([HIDDEN, SB * 128], f32, tag="selfq")
                nc.sync.dma_start(
                    out=selfq[:, :nodes], in_=qdram[:, b0 * 128 : b0 * 128 + nodes]
                )
                msg = mp.tile([128, NI_CAP], f32, tag="msg")
                nc.gpsimd.ap_gather(
                    out_ap=msg[:, :ni],
                    in_ap=table[:],
                    idxs_ap=idx_sb[:, int(colbase[s]) : int(colbase[s + 1])],
                    channels=128,
                    num_elems=2 * NPC,
                    d=1,
                    num_idxs=ni,
                )
                part = wp.tile([128, SB * 128], f32, tag="part")
                for k in range(nblk_s):
                    D = Ds[k]
                    cb = cbs[k]
                    nc.vector.tensor_reduce(
                        out=part[:, k * 128 : (k + 1) * 128],
                        in_=msg[:, cb : cb + 128 * D].rearrange(
                            "p (n d) -> p n d", d=D
                        ),
                        axis=mybir.AxisListType.X,
                        op=mybir.AluOpType.add,
                    )
                # self-loop contribution: add the node's own q into one
                # group's partial rows (the cross-group matmul sums over all
                # 8 groups, so any one group works)
                nc.vector.tensor_tensor(
                    out=part[0:16, :nodes],
                    in0=part[0:16, :nodes],
                    in1=selfq[:, :nodes],
                    op=mybir.AluOpType.add,
                )
                return part, b0, nblk_s

            # ---- layer 1 aggregation -> q2 shard --------------------------
            for s in range(len(supers)):
                part, b0, nblk_s = aggregate(s, q1d)
                psX = pp.tile([128, SB * HIDDEN], f32, tag="psX")
                for j in range(nblk_s):
                    nc.tensor.matmul(
                        out=psX[:, j * HIDDEN : (j + 1) * HIDDEN],
                        lhsT=part[:, j * 128 : (j + 1) * 128],
                        rhs=E8I[:],
                        start=True,
                        stop=True,
                    )
                qa4 = wp.tile([128, SB * HIDDEN], f32, tag="qa4")
                dslice = dinvr[:, b0 * HIDDEN : b0 * HIDDEN + nblk_s * HIDDEN]
                ql = qa4[:, : nblk_s * HIDDEN]
                nc.vector.tensor_tensor(
                    out=ql, in0=psX[:, : nblk_s * HIDDEN], in1=dslice,
                    op=mybir.AluOpType.mult,
                )
                nc.vector.tensor_tensor(
                    out=ql, in0=ql, in1=b1r[:, : nblk_s * HIDDEN],
                    op=mybir.AluOpType.add,
                )
                nc.vector.tensor_scalar_max(out=ql, in0=ql, scalar1=0.0)
                nc.vector.tensor_tensor(
                    out=ql, in0=ql, in1=dslice, op=mybir.AluOpType.mult
                )
                if b0 + nblk_s == NBLK:  # kill dummy nodes (last block tail)
                    sl = qa4[:, (nblk_s - 1) * HIDDEN : nblk_s * HIDDEN]
                    nc.vector.tensor_scalar_mul(out=sl, in0=sl, scalar1=dmask[:, :1])
                post_to_dram(qa4, b0, nblk_s, q2d)

            # ---- allgather 2 + table load ---------------------------------
            nc.gpsimd.collective_compute(
                "AllGather",
                mybir.AluOpType.bypass,
                replica_groups=rg,
                ins=[q2d[:]],
                outs=[tab2d[:]],
            )
            load_table(tab2d)

            # ---- layer 2 aggregation -> logits -> log_softmax -------------
            for s in range(len(supers)):
                part, b0, nblk_s = aggregate(s, q2d)
                psO = ppO.tile([128, SB * N_CLASSES], f32, tag="psO")
                for j in range(nblk_s):
                    nc.tensor.matmul(
                        out=psO[:, j * N_CLASSES : (j + 1) * N_CLASSES],
                        lhsT=part[:, j * 128 : (j + 1) * 128],
                        rhs=W2r[:],
                        start=True,
                        stop=True,
                    )
                z4 = wp.tile([128, SB * N_CLASSES], f32, tag="z4")
                for j in range(nblk_s):
                    b = b0 + j
                    nc.vector.tensor_scalar_mul(
                        out=z4[:, j * N_CLASSES : (j + 1) * N_CLASSES],
                        in0=psO[:, j * N_CLASSES : (j + 1) * N_CLASSES],
                        scalar1=dinvr[:, b * HIDDEN : b * HIDDEN + 1],
                    )
                zl = z4[:, : nblk_s * N_CLASSES]
                nc.vector.tensor_tensor(
                    out=zl, in0=zl, in1=b2r[:, : nblk_s * N_CLASSES],
                    op=mybir.AluOpType.add,
                )
                negm = wp.tile([128, SB], f32, tag="negm")
                nc.vector.tensor_reduce(
                    out=negm[:, :nblk_s],
                    in_=zl.rearrange("p (n c) -> p n c", c=N_CLASSES),
                    axis=mybir.AxisListType.X,
                    op=mybir.AluOpType.max,
                    negate=True,
                )
                e4 = wp.tile([128, SB * N_CLASSES], f32, tag="e4")
                ssum = wp.tile([128, SB], f32, tag="ssum")
                for j in range(nblk_s):
                    nc.scalar.activation(
                        out=e4[:, j * N_CLASSES : (j + 1) * N_CLASSES],
                        in_=z4[:, j * N_CLASSES : (j + 1) * N_CLASSES],
                        func=mybir.ActivationFunctionType.Exp,
                        bias=negm[:, j : j + 1],
                        scale=1.0,
                        accum_out=ssum[:, j : j + 1],
                    )
                ls = wp.tile([128, SB], f32, tag="ls")
                nc.scalar.activation(
                    out=ls[:, :nblk_s],
                    in_=ssum[:, :nblk_s],
                    func=mybir.ActivationFunctionType.Ln,
                )
                o4 = wp.tile([128, SB * N_CLASSES], f32, tag="o4")
                for j in range(nblk_s):
                    nc.vector.tensor_scalar(
                        out=o4[:, j * N_CLASSES : (j + 1) * N_CLASSES],
                        in0=z4[:, j * N_CLASSES : (j + 1) * N_CLASSES],
                        scalar1=negm[:, j : j + 1],
                        scalar2=ls[:, j : j + 1],
                        op0=mybir.AluOpType.add,
                        op1=mybir.AluOpType.subtract,
                    )
                for j in range(nblk_s):
                    nc.sync.dma_start(
                        out=out_d[b0 + j],
                        in_=o4[:, j * N_CLASSES : (j + 1) * N_CLASSES],
                    )

    nc.finalize()
    return nc


# ----------------------------------------------------------------------------
# entry point
# ----------------------------------------------------------------------------

def kernel(x, edge_index, W1, b1, W2, b2, _trace=False):
    x = np.asarray(x)
    edge_index = np.asarray(edge_index)
    W1 = np.asarray(W1, dtype=np.float32)
    b1 = np.asarray(b1, dtype=np.float32)
    W2 = np.asarray(W2, dtype=np.float32)
    b2 = np.asarray(b2, dtype=np.float32)

    if "meta" not in _cache:
        _cache["meta"] = _preprocess(edge_index)
        _cache["nc"] = _build_program(_cache["meta"])
    meta = _cache["meta"]
    nc = _cache["nc"]
    order = meta["order"]

    W1r = (
        W1.reshape(4, 128, HIDDEN).transpose(1, 0, 2).reshape(128, 64).astype(
            np.float32
        )
    )
    b1r = np.tile(b1, (128, SB)).astype(np.float32)
    b2r = np.tile(b2, (128, SB)).astype(np.float32)
    f_idx = np.arange(128) % HIDDEN
    E8I = np.eye(HIDDEN, dtype=np.float32)[f_idx]          # [128, 16]
    W2r = W2[f_idx].astype(np.float32)                      # [128, 64]
    ident = np.eye(128, dtype=np.float32)
    dmask = np.ones((128, 1), dtype=np.float32)
    dmask[128 - (NPC - NPC_REAL) :] = 0.0

    in_maps = []
    for c in range(NCORES):
        lo = c * NPC_REAL
        xc = np.zeros((NPC, N_FEAT), dtype=np.float32)
        real = order[c] < NPC_REAL
        xc[real] = x[lo + order[c][real]]
        in_maps.append(
            {
                "xT": np.ascontiguousarray(xc.T),
                "idx_in": meta["idx_all"][c],
                "degrep_in": meta["deg_rep"][c],
                "W1r": W1r,
                "b1r": b1r,
                "E8I": E8I,
                "W2r": W2r,
                "b2r": b2r,
                "ident": ident,
                "dmask": dmask,
            }
        )

    res = run_bass_kernel_spmd(nc, in_maps, list(range(NCORES)), trace=_trace)
    _cache["last_res"] = res

    out = np.empty((N_NODES, N_CLASSES), dtype=np.float32)
    for c in range(NCORES):
        oc = res.results[c]["out"].reshape(NPC, N_CLASSES)  # position-major
        lo = c * NPC_REAL
        real = order[c] < NPC_REAL
        out[lo + order[c][real]] = oc[real]
    return out
